# revision 1
# baseline (speedup 1.0000x reference)
"""Causal self-attention (B=2, T=2048, C=2048, H=16, rope) on 8 trn2 cores.

Sharding: tensor-parallel over heads. Each core owns 2 of 16 heads:
  - Wqkv columns for its heads (q,k,v), Wproj rows for its heads.
  - Computes qkv projection, rope, causal attention, and its partial
    output projection y_c = O_c @ Wproj_c  (full [4096, 2048]).
  - Host sums the 8 partials (the all-reduce / unshard for row-parallel TP).

All matmul operands fp16 (PE full rate; fp32 is 1/4 rate), fp32 PSUM
accumulation. Layouts keep the pipeline transpose-free except one 128x128
PE transpose per output tile (O -> O^T for the projection). Softmax sums
come free as a ones-column appended to V; normalization is applied to O
before projection. exp runs on the scalar engine over paired 2-bank PSUM
tiles; diagonal tiles are restricted to their valid causal column range.
"""

import sys

for _p in ("/opt/trn_rl_repo",):
    if _p not in sys.path:
        sys.path.append(_p)

import numpy as np

# ---- problem constants (hardcoded per the task contract) ----
B, T, C, H = 2, 2048, 2048, 16
D = C // H  # 128
NCORES = 8
HPC = H // NCORES  # heads per core = 2
NTOK = B * T  # 4096
P = 128
CT = C // P  # 16 contraction tiles
TOK512 = NTOK // 512  # 8
NQ = T // 512  # q-tiles per unit = 4
TT = NTOK // P  # 32 token 128-tiles
SCALE = 1.0 / np.sqrt(D)

_compiled = None

# tuning knobs (overridable before _build_bass for experiments)
KNOBS = {"cs": 2, "co": 4, "work": 2, "pt": 6, "ysb": 8, "rope": 4, "osb": 8, "xt": 2, "dact": 2, "oact": 1, "otact": 1}


def _build_bass():
    import concourse.bacc as bacc
    import concourse.mybir as mybir
    import concourse.tile as tile
    from contextlib import ExitStack

    f16 = mybir.dt.float16
    f32 = mybir.dt.float32
    Exp = mybir.ActivationFunctionType.Exp

    nc = bacc.Bacc()

    xT = nc.declare_dram_parameter("xT", [C, NTOK], f16, isOutput=False)
    wqk = nc.declare_dram_parameter("wqk", [C, 2 * HPC * D], f16, isOutput=False)
    wv = nc.declare_dram_parameter("wv", [C, HPC * D], f16, isOutput=False)
    wproj = nc.declare_dram_parameter("wproj", [HPC * D, C], f16, isOutput=False)
    cos_t = nc.declare_dram_parameter("cos_t", [P, NTOK], f16, isOutput=False)
    sin_t = nc.declare_dram_parameter("sin_t", [P, NTOK], f16, isOutput=False)
    maskw = nc.declare_dram_parameter("maskw", [P, 1024], f16, isOutput=False)
    ident = nc.declare_dram_parameter("ident", [P, P], f16, isOutput=False)
    rotmp = nc.declare_dram_parameter("rotm", [P, P], f16, isOutput=False)
    y = nc.declare_dram_parameter("y", [NTOK, C], f16, isOutput=True)

    with tile.TileContext(nc) as tc, ExitStack() as ctx:
        pers = ctx.enter_context(tc.tile_pool(name="pers", bufs=1))

        # ---- persistent SBUF tensors ----
        wqk_sb = pers.tile([P, CT, 4 * P], f16)  # [c128, ct, (q0,q1,k0,k1)*128]
        wv_sb = pers.tile([P, CT, 2 * P], f16)
        wproj_sb = pers.tile([P, HPC, C], f16)
        cos_sb = pers.tile([P, NTOK], f16)
        sin_sb = pers.tile([P, NTOK], f16)
        mask_sb = pers.tile([P, 1024], f16)
        id_sb = pers.tile([P, P], f16)
        rotm_sb = pers.tile([P, P], f16)
        qT_sb = pers.tile([P, HPC, NTOK], f16)  # [d, h, tok] rope'd
        kT_sb = pers.tile([P, HPC, NTOK], f16)
        v_sb = pers.tile([P, TT, HPC, D + 1], f16)  # [tokmod, tt, h, D|ones]
        oT_sb = pers.tile([P, TT, HPC, P], f16)  # [d, tt, h, tokmod]

        # ---- working pools (all open for the whole kernel: the stack
        # allocator must never reuse a released zone — released-zone deps
        # blow past the 1-wait/instruction HW limit pre-bacc-split) ----
        xt_pool = ctx.enter_context(tc.tile_pool(name="xt", bufs=KNOBS["xt"]))
        rope_pool = ctx.enter_context(tc.tile_pool(name="rope", bufs=KNOBS["rope"]))
        p_pool = ctx.enter_context(tc.tile_pool(name="pt", bufs=KNOBS["pt"]))
        osb_pool = ctx.enter_context(tc.tile_pool(name="osb", bufs=KNOBS["osb"]))
        ysb_pool = ctx.enter_context(tc.tile_pool(name="ysb", bufs=KNOBS["ysb"]))
        # PSUM (8 banks), phase-dedicated to avoid cross-phase slot stalls:
        #   work: 1-bank x3 (B: qk/rot/v chains; D: yps)
        #   cs:   1-bank x3 (C: S-tiles + transposes) — 3 slots decouple the
        #         PE from exp latency by one extra stage
        #   co:   1-bank x2 (C: packed O accumulators, 2 subs each)
        work_pool = ctx.enter_context(tc.tile_pool(name="work", bufs=KNOBS["work"], space="PSUM"))
        cs_pool = ctx.enter_context(tc.tile_pool(name="cs", bufs=KNOBS["cs"], space="PSUM"))
        co_pool = ctx.enter_context(tc.tile_pool(name="co", bufs=KNOBS["co"], space="PSUM"))

        for cth in range(2):
            nc.sync.dma_start(
                wqk_sb[:, cth * 8 : (cth + 1) * 8, :],
                wqk[cth * 1024 : (cth + 1) * 1024, :].rearrange(
                    "(ct p) m -> p ct m", p=P
                ),
            )
        nc.sync.dma_start(wv_sb[:], wv.rearrange("(ct p) m -> p ct m", p=P))
        nc.sync.dma_start(mask_sb[:], maskw[:])
        nc.sync.dma_start(id_sb[:], ident[:])
        nc.sync.dma_start(rotm_sb[:], rotmp[:])
        nc.vector.memset(v_sb[:, :, :, D : D + 1], 1.0)

        # ======== phase B: qkv projection + rope ========
        for ti in range(TOK512):
            t0 = ti * 512
            xt = xt_pool.tile([P, CT, 512], f16, tag="xt")
            for ch in range(4):
                nc.sync.dma_start(
                    xt[:, ch * 4 : (ch + 1) * 4, :],
                    xT[ch * 512 : (ch + 1) * 512, t0 : t0 + 512].rearrange(
                        "(ct p) j -> p ct j", p=P
                    ),
                )
            # stream rope tables alongside
            nc.sync.dma_start(cos_sb[:, t0 : t0 + 512], cos_t[:, t0 : t0 + 512])
            nc.sync.dma_start(sin_sb[:, t0 : t0 + 512], sin_t[:, t0 : t0 + 512])
            # q,k columns: out^T orientation -> [col128, tok512]
            for ci in range(4):
                hh = ci % HPC
                dstT = qT_sb if ci < HPC else kT_sb
                ps = work_pool.tile([P, 512], f32, tag="work", name="psqk")
                for ct in range(CT):
                    nc.tensor.matmul(
                        ps[:],
                        wqk_sb[:, ct, ci * P : (ci + 1) * P],
                        xt[:, ct, :],
                        start=(ct == 0),
                        stop=(ct == CT - 1),
                    )
                st = rope_pool.tile([P, 512], f16, tag="st")
                nc.vector.tensor_copy(st[:], ps[:])
                # half-rotation via PE permutation matmul (no partition-
                # crossing DVE/DMA needed)
                ps2 = work_pool.tile([P, 512], f32, tag="work", name="psrot")
                nc.tensor.matmul(ps2[:], rotm_sb[:], st[:], start=True, stop=True)
                t1 = rope_pool.tile([P, 512], f16, tag="t1")
                t2 = rope_pool.tile([P, 512], f16, tag="t2")
                nc.vector.tensor_mul(t1[:], st[:], cos_sb[:, t0 : t0 + 512])
                nc.vector.tensor_mul(t2[:], ps2[:], sin_sb[:, t0 : t0 + 512])
                nc.vector.tensor_add(dstT[:, hh, t0 : t0 + 512], t1[:], t2[:])
            # v: natural [tok, D*2] orientation (lhsT = xT tile)
            for sub in range(4):
                vps = work_pool.tile([P, 2 * P], f32, tag="work", name="vps")
                for ct in range(CT):
                    nc.tensor.matmul(
                        vps[:],
                        xt[:, ct, sub * P : (sub + 1) * P],
                        wv_sb[:, ct, :],
                        start=(ct == 0),
                        stop=(ct == CT - 1),
                    )
                tt = ti * 4 + sub
                for h in range(HPC):
                    nc.vector.tensor_copy(
                        v_sb[:, tt, h, 0:D], vps[:, h * P : (h + 1) * P]
                    )

        # ======== phase C: causal attention per (b, h) unit ========
        # S^T per k-tile: [k128, q512]. Diagonal k-tiles restricted to the
        # valid causal column range [g, 512). O accumulators packed 2 subs
        # per 1-bank tile (cols 0 and 256).
        for b in range(B):
            for h in range(HPC):
                toff = b * T
                for qi in range(NQ):
                    q0 = toff + qi * 512
                    ndiag0 = qi * 4  # first diagonal kt
                    nkt = ndiag0 + 4
                    # one accumulator per PSUM bank: two interleaved
                    # accumulation groups sharing a bank lose terms on HW
                    o_tiles = [
                        co_pool.tile([P, D + 1], f32, tag="co", name=f"o{_s}")
                        for _s in range(4)
                    ]

                    def pv(pt_ap, kt, sub_lo):
                        for s in range(sub_lo, 4):
                            nc.tensor.matmul(
                                o_tiles[s][:],
                                pt_ap(s),
                                v_sb[:, b * 16 + kt, h, :],
                                start=(kt == 0),
                                stop=(kt == ndiag0 + s),
                            )

                    for kt in range(nkt):
                        k0 = toff + kt * P
                        gi = kt - ndiag0
                        g = max(gi, 0) * P
                        w = 512 - g
                        sd = cs_pool.tile([P, 512], f32, tag="cs", name="sd")
                        nc.tensor.matmul(
                            sd[:, 0:w],
                            kT_sb[:, h, k0 : k0 + P],
                            qT_sb[:, h, q0 + g : q0 + 512],
                            start=True,
                            stop=True,
                        )
                        ptd = p_pool.tile([P, 512], f16, tag="pt", name="ptd")
                        nc.scalar.activation(
                            ptd[:, 0:w], sd[:, 0:w], Exp, scale=float(SCALE)
                        )
                        if gi >= 0:  # diagonal: multiplicative causal mask
                            nc.vector.tensor_mul(
                                ptd[:, 0:w], ptd[:, 0:w], mask_sb[:, 384 : 384 + w]
                            )
                        pv(
                            lambda s, _g=g: ptd[:, s * P - _g : s * P - _g + P],
                            kt,
                            max(gi, 0),
                        )
                    # drain: normalize O rows by 1/rowsum, transpose to O^T
                    for sub in range(4):
                        tt = b * 16 + qi * 4 + sub
                        ot = o_tiles[sub]
                        rtmp = osb_pool.tile([P, 1], f32, tag="rtmp")
                        nc.vector.reciprocal(rtmp[:], ot[:, D : D + 1])
                        o_sb = osb_pool.tile([P, P], f16, tag="osb")
                        if KNOBS["oact"] and sub % 2 == 1:
                            nc.scalar.mul(o_sb[:], ot[:, 0:D], rtmp[:])
                        else:
                            nc.vector.tensor_scalar_mul(o_sb[:], ot[:, 0:D], rtmp[:])
                        tp = cs_pool.tile([P, P], f16, tag="cs", name="tp")
                        nc.tensor.transpose(tp[:], o_sb[:], id_sb[:])
                        if KNOBS["otact"] and sub % 2 == 0:
                            nc.scalar.copy(oT_sb[:, tt, h, :], tp[:])
                        else:
                            nc.vector.tensor_copy(oT_sb[:, tt, h, :], tp[:])

        # deferred wproj load (only needed for phase D)
        nc.sync.dma_start(wproj_sb[:], wproj.rearrange("(h p) m -> p h m", p=P))

        # deferred wproj load (only needed for phase D)
        nc.sync.dma_start(wproj_sb[:], wproj.rearrange("(h p) m -> p h m", p=P))

        # ======== phase D: output projection ========
        for tt in range(TT):
            for cc in range(4):
                yps = work_pool.tile([P, 512], f32, tag="work", name="yps")
                for h in range(HPC):
                    nc.tensor.matmul(
                        yps[:],
                        oT_sb[:, tt, h, :],
                        wproj_sb[:, h, cc * 512 : (cc + 1) * 512],
                        start=(h == 0),
                        stop=(h == HPC - 1),
                    )
                ysb = ysb_pool.tile([P, 512], f16, tag="ysb")
                if cc % KNOBS["dact"] == 0:
                    nc.scalar.copy(ysb[:], yps[:])
                else:
                    nc.vector.tensor_copy(ysb[:], yps[:])
                nc.sync.dma_start(
                    y[tt * P : (tt + 1) * P, cc * 512 : (cc + 1) * 512], ysb[:]
                )

    # bacc lowering: splits multi-sem waits into EventSemaphore insts
    # (TRN2 allows at most 1 wait per regular instruction), reg alloc, DCE.
    nc.compile()
    return nc


def _host_inputs(x, Wqkv, Wproj):
    """Build per-core device input maps (host-side sharding)."""
    xTf = np.ascontiguousarray(x.reshape(NTOK, C).T).astype(np.float16)

    invf = 1.0 / (10000.0 ** (np.arange(0, D, 2, dtype=np.float32) / D))
    freqs = np.arange(T, dtype=np.float32)[:, None] * invf[None, :]  # [T, 64]
    cos = np.cos(freqs).astype(np.float32).T  # [64, T]
    sin = np.sin(freqs).astype(np.float32).T
    cos_t = np.tile(np.concatenate([cos, cos], axis=0), (1, B)).astype(np.float16)
    sin_t = np.tile(np.concatenate([-sin, sin], axis=0), (1, B)).astype(np.float16)

    ii = np.arange(P)[:, None]
    mm = np.arange(1024)[None, :]
    maskw = (mm >= ii + 384).astype(np.float16)
    ident = np.eye(P, dtype=np.float16)
    rotm = np.zeros((P, P), dtype=np.float16)
    rotm[(np.arange(P) + 64) % P, np.arange(P)] = 1.0

    in_maps = []
    for c in range(NCORES):
        h0 = c * HPC * D  # col offset of this core's heads
        wqk_c = np.concatenate(
            [Wqkv[:, h0 : h0 + HPC * D], Wqkv[:, C + h0 : C + h0 + HPC * D]], axis=1
        ).astype(np.float16)
        wv_c = Wqkv[:, 2 * C + h0 : 2 * C + h0 + HPC * D].astype(np.float16)
        wproj_c = np.ascontiguousarray(Wproj[h0 : h0 + HPC * D, :]).astype(np.float16)
        in_maps.append(
            {
                "xT": xTf,
                "wqk": np.ascontiguousarray(wqk_c),
                "wv": np.ascontiguousarray(wv_c),
                "wproj": wproj_c,
                "cos_t": cos_t,
                "sin_t": sin_t,
                "maskw": maskw,
                "ident": ident,
                "rotm": rotm,
            }
        )
    return in_maps


def kernel(x, Wqkv, Wproj, _trace=False):
    global _compiled
    x = np.asarray(x, dtype=np.float32)
    Wqkv = np.asarray(Wqkv, dtype=np.float32)
    Wproj = np.asarray(Wproj, dtype=np.float32)

    from concourse.bass_utils import run_bass_kernel_spmd

    if _compiled is None:
        _compiled = _build_bass()
    nc = _compiled

    in_maps = _host_inputs(x, Wqkv, Wproj)
    res = run_bass_kernel_spmd(nc, in_maps, list(range(NCORES)), trace=_trace)
    out = np.zeros((NTOK, C), dtype=np.float32)
    for r in res.results:
        out += r["y"].astype(np.float32)
    kernel._last_result = res
    return out.reshape(B, T, C)



# revision 19
# speedup vs baseline: 1.0962x; 1.0962x over previous
"""Causal self-attention (B=2, T=2048, C=2048, H=16, rope) on 8 trn2 cores.

Sharding: tensor-parallel over heads (2 heads/core); host sums the 8
row-parallel partial output projections.

v2: fp8 (e4m3) hi/lo-split DoubleRow matmuls for the QKV and output
projections (3-term x_hi*w_hi + x_lo*w_hi + x_hi*w_lo, fp32 PSUM accum,
~1.4e-3 rel err), attention core in fp16. Weights pre-scaled x32 on host
so fp8 residuals stay in normal range; output rescaled on host.
Engine routing: Act = exp only; DVE = rope/normalize/half the y copies;
Pool(gpsimd) = O hi/lo quantize + other half of y copies. Phases are
interleaved (B(b1) into C(b0), D(b0) into C(b1)) to keep the PE fed
while the Act engine works through the exp backlog.
"""

import sys

for _p in ("/opt/trn_rl_repo",):
    if _p not in sys.path:
        sys.path.append(_p)

import numpy as np

# ---- problem constants (hardcoded per the task contract) ----
B, T, C, H = 2, 2048, 2048, 16
D = C // H  # 128
NCORES = 8
HPC = H // NCORES  # heads per core = 2
NTOK = B * T  # 4096
P = 128
CT = C // P  # 16 contraction tiles
TOK512 = NTOK // 512  # 8
NQ = T // 512  # q-tiles per unit = 4
TT = NTOK // P  # 32 token 128-tiles
SCALE = 1.0 / np.sqrt(D)
WSCALE = 32.0  # host pre-scale on all weights (fp8 residual range)
ONESC = 8.0  # ones-column value: o_sb = (32/ONESC) * O_normalized
YDIV = WSCALE * WSCALE / ONESC  # host divides y by this

_compiled = None

KNOBS = {"pt": 8, "ysb": 12, "rope": 4, "osb": 8, "xt": 2, "dfill": 4}


def _build_bass():
    import concourse.bacc as bacc
    import concourse.mybir as mybir
    import concourse.tile as tile
    from contextlib import ExitStack

    f16 = mybir.dt.float16
    f32 = mybir.dt.float32
    f8 = mybir.dt.float8e4
    DR = mybir.MatmulPerfMode.DoubleRow
    Exp = mybir.ActivationFunctionType.Exp

    nc = bacc.Bacc()

    xhi = nc.declare_dram_parameter("xhi", [C, NTOK], f8, isOutput=False)
    xlo = nc.declare_dram_parameter("xlo", [C, NTOK], f8, isOutput=False)
    wqkhi = nc.declare_dram_parameter("wqkhi", [C, 2 * HPC * D], f8, isOutput=False)
    wqklo = nc.declare_dram_parameter("wqklo", [C, 2 * HPC * D], f8, isOutput=False)
    wvhi = nc.declare_dram_parameter("wvhi", [C, HPC * D], f8, isOutput=False)
    wvlo = nc.declare_dram_parameter("wvlo", [C, HPC * D], f8, isOutput=False)
    wphi = nc.declare_dram_parameter("wphi", [HPC * D, C], f8, isOutput=False)
    wplo = nc.declare_dram_parameter("wplo", [HPC * D, C], f8, isOutput=False)
    cos_t = nc.declare_dram_parameter("cos_t", [P, NTOK], f16, isOutput=False)
    sin_t = nc.declare_dram_parameter("sin_t", [P, NTOK], f16, isOutput=False)
    maskw = nc.declare_dram_parameter("maskw", [P, 1024], f16, isOutput=False)
    ident = nc.declare_dram_parameter("ident", [P, P], f16, isOutput=False)
    rotmp = nc.declare_dram_parameter("rotm", [P, P], f16, isOutput=False)
    y = nc.declare_dram_parameter("y", [NTOK, C], f16, isOutput=True)

    with tile.TileContext(nc) as tc, ExitStack() as ctx:
        pers = ctx.enter_context(tc.tile_pool(name="pers", bufs=1))

        # ---- persistent SBUF tensors ----
        wqkh_sb = pers.tile([P, CT, 4 * P], f8)
        wqkl_sb = pers.tile([P, CT, 4 * P], f8)
        wvh_sb = pers.tile([P, CT, 2 * P], f8)
        wvl_sb = pers.tile([P, CT, 2 * P], f8)
        wph_sb = pers.tile([P, HPC, C], f8)
        wpl_sb = pers.tile([P, HPC, C], f8)
        cos_sb = pers.tile([P, NTOK], f16)
        sin_sb = pers.tile([P, NTOK], f16)
        mask_sb = pers.tile([P, 1024], f16)
        id_sb = pers.tile([P, P], f16)
        rotm_sb = pers.tile([P, P], f16)
        qT_sb = pers.tile([P, HPC, NTOK], f16)  # [d, h, tok] rope'd, 32x scale
        kT_sb = pers.tile([P, HPC, NTOK], f16)
        v_sb = pers.tile([P, TT, HPC, D + 1], f16)  # [tokmod, tt, h, D|ones*8]
        oTh_sb = pers.tile([P, TT, HPC, P], f8)  # [d, tt, h, tokmod] hi
        oTl_sb = pers.tile([P, TT, HPC, P], f8)  # lo residual

        xt_pool = ctx.enter_context(tc.tile_pool(name="xt", bufs=KNOBS["xt"]))
        rope_pool = ctx.enter_context(tc.tile_pool(name="rope", bufs=KNOBS["rope"]))
        p_pool = ctx.enter_context(tc.tile_pool(name="pt", bufs=KNOBS["pt"]))
        osb_pool = ctx.enter_context(tc.tile_pool(name="osb", bufs=KNOBS["osb"]))
        ysb_pool = ctx.enter_context(tc.tile_pool(name="ysb", bufs=KNOBS["ysb"]))
        # Single PSUM pool, 8 banks via per-tag bufs:
        #   ta(2): qk+v DR chains (B), S tiles + O transposes (C)
        #   tb(2): rope rot matmuls (B), yps projection chains (D)
        #   co(4): O accumulators (C)
        ps_pool = ctx.enter_context(tc.tile_pool(name="ps", bufs=1, space="PSUM"))

        def pstile(tag, bufs, shape=(P, 512), dt=f32, name="ps"):
            return ps_pool.tile(list(shape), dt, tag=tag, bufs=bufs, name=name)

        # startup DMAs ordered by first use
        for cth in range(2):
            nc.sync.dma_start(
                wqkh_sb[:, cth * 8 : (cth + 1) * 8, :],
                wqkhi[cth * 1024 : (cth + 1) * 1024, :].rearrange(
                    "(ct p) m -> p ct m", p=P
                ),
            )

        first_tile_loaded = [False]

        def load_rest_weights():
            for cth in range(2):
                nc.sync.dma_start(
                    wqkl_sb[:, cth * 8 : (cth + 1) * 8, :],
                    wqklo[cth * 1024 : (cth + 1) * 1024, :].rearrange(
                        "(ct p) m -> p ct m", p=P
                    ),
                )

        # ======== phase B: qkv projection + rope (fp8 hi/lo DR) ========
        def emit_B_tile(ti):
            t0 = ti * 512
            xh = xt_pool.tile([P, CT, 512], f8, tag="xh", name="xh")
            xl = xt_pool.tile([P, CT, 512], f8, tag="xl", name="xl")
            for ch in range(4):
                nc.sync.dma_start(
                    xh[:, ch * 4 : (ch + 1) * 4, :],
                    xhi[ch * 512 : (ch + 1) * 512, t0 : t0 + 512].rearrange(
                        "(ct p) j -> p ct j", p=P
                    ),
                )
            if not first_tile_loaded[0]:
                first_tile_loaded[0] = True
                load_rest_weights()
            for ch in range(4):
                nc.sync.dma_start(
                    xl[:, ch * 4 : (ch + 1) * 4, :],
                    xlo[ch * 512 : (ch + 1) * 512, t0 : t0 + 512].rearrange(
                        "(ct p) j -> p ct j", p=P
                    ),
                )
            if ti == 0:
                nc.sync.dma_start(wvh_sb[:], wvhi.rearrange("(ct p) m -> p ct m", p=P))
                nc.sync.dma_start(wvl_sb[:], wvlo.rearrange("(ct p) m -> p ct m", p=P))
                nc.sync.dma_start(rotm_sb[:], rotmp[:])
                nc.vector.memset(v_sb[:, :, :, D : D + 1], float(ONESC))
            nc.sync.dma_start(cos_sb[:, t0 : t0 + 512], cos_t[:, t0 : t0 + 512])
            nc.sync.dma_start(sin_sb[:, t0 : t0 + 512], sin_t[:, t0 : t0 + 512])
            if ti == 1:
                nc.sync.dma_start(mask_sb[:], maskw[:])
                nc.sync.dma_start(id_sb[:], ident[:])
            # v chains are split into halves and woven into the rope rot
            # gaps: after rot_h0 the PE does half a v chain while the DVE/Pool
            # finish st_h1, so rot_h1 never stalls the PE.
            vstate = {"sub": 0, "vps": None, "piece": 0}

            def v_piece():
                sub = vstate["sub"]
                if sub >= 4:
                    return
                if vstate["piece"] == 0:
                    vstate["vps"] = pstile("ta", 2, name="vps")
                vps = vstate["vps"]
                base = vstate["piece"] * 12
                i = 0
                terms = [
                    (xsb, wsb, c)
                    for xsb, wsb in ((xh, wvh_sb), (xl, wvh_sb), (xh, wvl_sb))
                    for c in range(0, CT, 2)
                ]
                for i in range(base, base + 12):
                    xsb, wsb, c = terms[i]
                    nc.tensor.matmul(
                        vps[:, 0:256],
                        xsb[:, c : c + 2, sub * P : (sub + 1) * P],
                        wsb[:, c : c + 2, :],
                        start=(i == 0),
                        stop=(i == 23),
                        perf_mode=DR,
                    )
                if vstate["piece"] == 1:
                    tt = ti * 4 + sub
                    for h in range(HPC):
                        nc.vector.tensor_copy(
                            v_sb[:, tt, h, 0:D], vps[:, h * P : (h + 1) * P]
                        )
                    vstate["sub"] += 1
                vstate["piece"] ^= 1

            # q,k columns: out^T orientation -> [col128, tok256].
            for ci in range(4):
                hh = ci % HPC
                dstT = qT_sb if ci < HPC else kT_sb
                sts = []
                for half in range(2):
                    j0 = half * 256
                    ps = pstile("ta", 2, name="psqk")
                    i = 0
                    for wsb, xsb in ((wqkh_sb, xh), (wqkl_sb, xh), (wqkh_sb, xl)):
                        for c in range(0, CT, 2):
                            nc.tensor.matmul(
                                ps[:, 0:256],
                                wsb[:, c : c + 2, ci * P : (ci + 1) * P],
                                xsb[:, c : c + 2, j0 : j0 + 256],
                                start=(i == 0),
                                stop=(i == 23),
                                perf_mode=DR,
                            )
                            i += 1
                    st = rope_pool.tile([P, 256], f16, tag=f"st{half}", name="st")
                    if half == 0:
                        nc.vector.tensor_copy(st[:], ps[:, 0:256])
                    else:
                        nc.scalar.copy(st[:], ps[:, 0:256])
                    sts.append(st)
                for half in range(2):
                    j0 = half * 256
                    st = sts[half]
                    ps2 = pstile("tb", 2, name="psrot")
                    nc.tensor.matmul(
                        ps2[:, 0:256], rotm_sb[:], st[:], start=True, stop=True
                    )
                    t1 = rope_pool.tile([P, 256], f16, tag=f"t1{half}", name="t1")
                    t2 = rope_pool.tile([P, 256], f16, tag=f"t2{half}", name="t2")
                    nc.vector.tensor_mul(t1[:], st[:], cos_sb[:, t0 + j0 : t0 + j0 + 256])
                    nc.vector.tensor_mul(
                        t2[:], ps2[:, 0:256], sin_sb[:, t0 + j0 : t0 + j0 + 256]
                    )
                    nc.vector.tensor_add(
                        dstT[:, hh, t0 + j0 : t0 + j0 + 256], t1[:], t2[:]
                    )
                    if half == 0:
                        v_piece()
                v_piece()

        # ======== phase D chain supply (woven into C) ========
        dq = []  # pending (tt, cc, sub) projection chains
        ysb_map = {}

        d_alt = [0]

        def emit_d_chain(alt=False):
            if not dq:
                return False
            tt, cc, sub = dq.pop(0)
            if sub == 0:
                ysb_map[(tt, cc)] = ysb_pool.tile(
                    [P, 512], f16, tag="ysb", name="ysb"
                )
            ysb = ysb_map[(tt, cc)]
            c0 = cc * 512 + sub * 256
            if alt and d_alt[0] % 2 == 0:
                yps = pstile("ta", 2, name="yps2")
            else:
                yps = pstile("tb", 2, name="yps")
            d_alt[0] += 1
            i = 0
            for osb, wsb in ((oTh_sb, wph_sb), (oTl_sb, wph_sb), (oTh_sb, wpl_sb)):
                nc.tensor.matmul(
                    yps[:, 0:256],
                    osb[:, tt, :, :],
                    wsb[:, :, c0 : c0 + 256],
                    start=(i == 0),
                    stop=(i == 2),
                    perf_mode=DR,
                )
                i += 1
            act_share = 2 if alt else 8
            if (2 * cc + sub) % act_share == 0:
                nc.scalar.copy(ysb[:, sub * 256 : sub * 256 + 256], yps[:, 0:256])
            else:
                nc.vector.tensor_copy(ysb[:, sub * 256 : sub * 256 + 256], yps[:, 0:256])
            if sub == 1:
                nc.sync.dma_start(
                    y[tt * P : (tt + 1) * P, cc * 512 : (cc + 1) * 512], ysb[:]
                )
                del ysb_map[(tt, cc)]
            return True

        # ======== phase C: causal attention, one (b,h,qi) unit ========
        def emit_C(b, h, qi):
            toff = b * T
            q0 = toff + qi * 512
            ndiag0 = qi * 4
            nkt = ndiag0 + 4
            o_tiles = [
                pstile("co", 4, shape=(P, D + 1), name=f"o{_s}") for _s in range(4)
            ]

            def pv(pt_ap, kt, sub_lo):
                for s in range(sub_lo, 4):
                    nc.tensor.matmul(
                        o_tiles[s][:],
                        pt_ap(s),
                        v_sb[:, b * 16 + kt, h, :],
                        start=(kt == 0),
                        stop=(kt == ndiag0 + s),
                    )

            pend = []  # (ptd, kt, g) awaiting PV, emitted with lag 2

            def flush_pv():
                ptd_, kt_, g_ = pend.pop(0)
                pv(
                    lambda s, _p=ptd_, _g=g_: _p[:, s * P - _g : s * P - _g + P],
                    kt_,
                    max(kt_ - ndiag0, 0),
                )

            for kt in range(nkt):
                k0 = toff + kt * P
                gi = kt - ndiag0
                g = max(gi, 0) * P
                w = 512 - g
                sd = pstile("ta", 2, name="sd")
                nc.tensor.matmul(
                    sd[:, 0:w],
                    kT_sb[:, h, k0 : k0 + P],
                    qT_sb[:, h, q0 + g : q0 + 512],
                    start=True,
                    stop=True,
                )
                ptd = p_pool.tile([P, 512], f16, tag="pt", name="ptd")
                nc.scalar.activation(
                    ptd[:, 0:w], sd[:, 0:w], Exp,
                    scale=float(SCALE / (WSCALE * WSCALE)),
                )
                if gi >= 0:
                    nc.gpsimd.tensor_mul(
                        ptd[:, 0:w], ptd[:, 0:w], mask_sb[:, 384 : 384 + w]
                    )
                emit_d_chain()
                pend.append((ptd, kt, g))
                if len(pend) > 3:
                    flush_pv()
            while pend:
                emit_d_chain()
                flush_pv()
            # drain: normalize, transpose, fp8 hi/lo quantize; D chains fill
            # the DVE-latency bubbles between subs
            for sub in range(4):
                tt = b * 16 + qi * 4 + sub
                ot = o_tiles[sub]
                rtmp = osb_pool.tile([P, 1], f32, tag="rtmp", name="rtmp")
                nc.vector.reciprocal(rtmp[:], ot[:, D : D + 1])
                o_sb = osb_pool.tile([P, P], f16, tag="osb", name="osb")
                nc.vector.tensor_scalar_mul(o_sb[:], ot[:, 0:D], rtmp[:])
                emit_d_chain()
                tp = pstile("tb", 2, shape=(P, P), dt=f16, name="tp")
                nc.tensor.transpose(tp[:], o_sb[:], id_sb[:])
                oT16 = osb_pool.tile([P, P], f16, tag="oT16", name="oT16")
                nc.vector.tensor_copy(oT16[:], tp[:])
                nc.gpsimd.tensor_copy(oTh_sb[:, tt, h, :], oT16[:])
                nc.gpsimd.tensor_sub(oTl_sb[:, tt, h, :], oT16[:], oTh_sb[:, tt, h, :])
                emit_d_chain()

        # ======== schedule ========
        for ti in range(TOK512):
            emit_B_tile(ti)
        nc.sync.dma_start(wph_sb[:], wphi.rearrange("(h p) m -> p h m", p=P))
        nc.sync.dma_start(wpl_sb[:], wplo.rearrange("(h p) m -> p h m", p=P))
        for b in range(B):
            for qi in range(NQ):
                emit_C(b, 0, qi)
                emit_C(b, 1, qi)
                dq.extend(
                    (b * 16 + qi * 4 + s, cc, sub)
                    for s in range(4)
                    for cc in range(4)
                    for sub in range(2)
                )
        while emit_d_chain(alt=True):
            pass

    nc.compile()
    return nc


def _host_inputs(x, Wqkv, Wproj):
    """Build per-core device input maps (host-side sharding + fp8 hi/lo)."""
    import ml_dtypes

    NP8 = ml_dtypes.float8_e4m3

    def hilo(a):
        hi = a.astype(NP8)
        lo = (a - hi.astype(np.float32)).astype(NP8)
        return np.ascontiguousarray(hi), np.ascontiguousarray(lo)

    xTf = np.ascontiguousarray(x.reshape(NTOK, C).T).astype(np.float32)
    xhi, xlo = hilo(xTf)

    invf = 1.0 / (10000.0 ** (np.arange(0, D, 2, dtype=np.float32) / D))
    freqs = np.arange(T, dtype=np.float32)[:, None] * invf[None, :]  # [T, 64]
    cos = np.cos(freqs).astype(np.float32).T  # [64, T]
    sin = np.sin(freqs).astype(np.float32).T
    cos_t = np.tile(np.concatenate([cos, cos], axis=0), (1, B)).astype(np.float16)
    sin_t = np.tile(np.concatenate([-sin, sin], axis=0), (1, B)).astype(np.float16)

    ii = np.arange(P)[:, None]
    mm = np.arange(1024)[None, :]
    maskw = (mm >= ii + 384).astype(np.float16)
    ident = np.eye(P, dtype=np.float16)
    rotm = np.zeros((P, P), dtype=np.float16)
    rotm[(np.arange(P) + 64) % P, np.arange(P)] = 1.0

    in_maps = []
    for c in range(NCORES):
        h0 = c * HPC * D
        wqk_c = np.concatenate(
            [Wqkv[:, h0 : h0 + HPC * D], Wqkv[:, C + h0 : C + h0 + HPC * D]], axis=1
        ).astype(np.float32) * WSCALE
        wv_c = Wqkv[:, 2 * C + h0 : 2 * C + h0 + HPC * D].astype(np.float32) * WSCALE
        wp_c = Wproj[h0 : h0 + HPC * D, :].astype(np.float32) * WSCALE
        wqkhi, wqklo = hilo(wqk_c)
        wvhi, wvlo = hilo(wv_c)
        wphi, wplo = hilo(wp_c)
        in_maps.append(
            {
                "xhi": xhi,
                "xlo": xlo,
                "wqkhi": wqkhi,
                "wqklo": wqklo,
                "wvhi": wvhi,
                "wvlo": wvlo,
                "wphi": wphi,
                "wplo": wplo,
                "cos_t": cos_t,
                "sin_t": sin_t,
                "maskw": maskw,
                "ident": ident,
                "rotm": rotm,
            }
        )
    return in_maps


def kernel(x, Wqkv, Wproj, _trace=False):
    global _compiled
    x = np.asarray(x, dtype=np.float32)
    Wqkv = np.asarray(Wqkv, dtype=np.float32)
    Wproj = np.asarray(Wproj, dtype=np.float32)

    from concourse.bass_utils import run_bass_kernel_spmd

    if _compiled is None:
        _compiled = _build_bass()
    nc = _compiled

    in_maps = _host_inputs(x, Wqkv, Wproj)
    res = run_bass_kernel_spmd(nc, in_maps, list(range(NCORES)), trace=_trace)
    out = np.zeros((NTOK, C), dtype=np.float32)
    for r in res.results:
        out += r["y"].astype(np.float32)
    out /= YDIV
    kernel._last_result = res
    return out.reshape(B, T, C)


# revision 25
# speedup vs baseline: 1.1225x; 1.0240x over previous
"""Causal self-attention (B=2, T=2048, C=2048, H=16, rope) on 8 trn2 cores.

Sharding: tensor-parallel over heads (2 heads/core); host sums the 8
row-parallel partial output projections.

v2: fp8 (e4m3) hi/lo-split DoubleRow matmuls for the QKV and output
projections (3-term x_hi*w_hi + x_lo*w_hi + x_hi*w_lo, fp32 PSUM accum,
~1.4e-3 rel err), attention core in fp16. Weights pre-scaled x32 on host
so fp8 residuals stay in normal range; output rescaled on host.
Engine routing (gpsimd/Pool cannot touch PSUM): Act = exp + one st copy
per ci + 1/8 of y copies; DVE = rope muls, PSUM drains (st/recip/
normalize/oT/y copies); Pool = SBUF-only work (causal mask muls, O hi/lo
fp8 quantize). Schedule: B tiles run standalone (PE-dense, v chains
woven into rope-rot latency gaps); attention units pipeline S two k-tiles
ahead of PV (lag-3 flush) and weave output-projection chains into the
exp-latency and drain bubbles; trailing projections alternate PSUM rings.
PSUM banks: ta(2)=qk/v/S/… tb(2)=rot/yps/transposes, co(4)=O accum.
"""

import sys

for _p in ("/opt/trn_rl_repo",):
    if _p not in sys.path:
        sys.path.append(_p)

import numpy as np

# ---- problem constants (hardcoded per the task contract) ----
B, T, C, H = 2, 2048, 2048, 16
D = C // H  # 128
NCORES = 8
HPC = H // NCORES  # heads per core = 2
NTOK = B * T  # 4096
P = 128
CT = C // P  # 16 contraction tiles
TOK512 = NTOK // 512  # 8
NQ = T // 512  # q-tiles per unit = 4
TT = NTOK // P  # 32 token 128-tiles
SCALE = 1.0 / np.sqrt(D)
WSCALE = 32.0  # host pre-scale on all weights (fp8 residual range)
ONESC = 8.0  # ones-column value: o_sb = (32/ONESC) * O_normalized
YDIV = WSCALE * WSCALE / ONESC  # host divides y by this

_compiled = None

KNOBS = {"pt": 8, "ysb": 12, "rope": 4, "osb": 8, "xt": 2, "dfill": 4}


def _build_bass():
    import concourse.bacc as bacc
    import concourse.mybir as mybir
    import concourse.tile as tile
    from contextlib import ExitStack

    f16 = mybir.dt.float16
    f32 = mybir.dt.float32
    f8 = mybir.dt.float8e4
    DR = mybir.MatmulPerfMode.DoubleRow
    Exp = mybir.ActivationFunctionType.Exp

    nc = bacc.Bacc()

    xhi = nc.declare_dram_parameter("xhi", [C, NTOK], f8, isOutput=False)
    xlo = nc.declare_dram_parameter("xlo", [C, NTOK], f8, isOutput=False)
    wqkhi = nc.declare_dram_parameter("wqkhi", [C, 2 * HPC * D], f8, isOutput=False)
    wqklo = nc.declare_dram_parameter("wqklo", [C, 2 * HPC * D], f8, isOutput=False)
    wvhi = nc.declare_dram_parameter("wvhi", [C, HPC * D], f8, isOutput=False)
    wvlo = nc.declare_dram_parameter("wvlo", [C, HPC * D], f8, isOutput=False)
    wphi = nc.declare_dram_parameter("wphi", [HPC * D, C], f8, isOutput=False)
    wplo = nc.declare_dram_parameter("wplo", [HPC * D, C], f8, isOutput=False)
    cos_t = nc.declare_dram_parameter("cos_t", [P, NTOK], f16, isOutput=False)
    sin_t = nc.declare_dram_parameter("sin_t", [P, NTOK], f16, isOutput=False)
    maskw = nc.declare_dram_parameter("maskw", [P, 1024], f16, isOutput=False)
    ident = nc.declare_dram_parameter("ident", [P, P], f16, isOutput=False)
    rotmp = nc.declare_dram_parameter("rotm", [P, P], f16, isOutput=False)
    y = nc.declare_dram_parameter("y", [NTOK, C], f16, isOutput=True)

    with tile.TileContext(nc) as tc, ExitStack() as ctx:
        pers = ctx.enter_context(tc.tile_pool(name="pers", bufs=1))

        # ---- persistent SBUF tensors ----
        wqkh_sb = pers.tile([P, CT, 4 * P], f8)
        wqkl_sb = pers.tile([P, CT, 4 * P], f8)
        wvh_sb = pers.tile([P, CT, 2 * P], f8)
        wvl_sb = pers.tile([P, CT, 2 * P], f8)
        wph_sb = pers.tile([P, HPC, C], f8)
        wpl_sb = pers.tile([P, HPC, C], f8)
        cos_sb = pers.tile([P, NTOK], f16)
        sin_sb = pers.tile([P, NTOK], f16)
        mask_sb = pers.tile([P, 1024], f16)
        id_sb = pers.tile([P, P], f16)
        rotm_sb = pers.tile([P, P], f16)
        qT_sb = pers.tile([P, HPC, NTOK], f16)  # [d, h, tok] rope'd, 32x scale
        kT_sb = pers.tile([P, HPC, NTOK], f16)
        v_sb = pers.tile([P, TT, HPC, D + 1], f16)  # [tokmod, tt, h, D|ones*8]
        oTh_sb = pers.tile([P, TT, HPC, P], f8)  # [d, tt, h, tokmod] hi
        oTl_sb = pers.tile([P, TT, HPC, P], f8)  # lo residual

        xt_pool = ctx.enter_context(tc.tile_pool(name="xt", bufs=KNOBS["xt"]))
        rope_pool = ctx.enter_context(tc.tile_pool(name="rope", bufs=KNOBS["rope"]))
        p_pool = ctx.enter_context(tc.tile_pool(name="pt", bufs=KNOBS["pt"]))
        osb_pool = ctx.enter_context(tc.tile_pool(name="osb", bufs=KNOBS["osb"]))
        ysb_pool = ctx.enter_context(tc.tile_pool(name="ysb", bufs=KNOBS["ysb"]))
        # Single PSUM pool, 8 banks via per-tag bufs:
        #   ta(2): qk+v DR chains (B), S tiles + O transposes (C)
        #   tb(2): rope rot matmuls (B), yps projection chains (D)
        #   co(4): O accumulators (C)
        ps_pool = ctx.enter_context(tc.tile_pool(name="ps", bufs=1, space="PSUM"))

        def pstile(tag, bufs, shape=(P, 512), dt=f32, name="ps"):
            return ps_pool.tile(list(shape), dt, tag=tag, bufs=bufs, name=name)

        # startup DMAs ordered by first use
        for cth in range(2):
            nc.sync.dma_start(
                wqkh_sb[:, cth * 8 : (cth + 1) * 8, :],
                wqkhi[cth * 1024 : (cth + 1) * 1024, :].rearrange(
                    "(ct p) m -> p ct m", p=P
                ),
            )

        first_tile_loaded = [False]

        def load_rest_weights():
            for cth in range(2):
                nc.sync.dma_start(
                    wqkl_sb[:, cth * 8 : (cth + 1) * 8, :],
                    wqklo[cth * 1024 : (cth + 1) * 1024, :].rearrange(
                        "(ct p) m -> p ct m", p=P
                    ),
                )

        # ======== phase B: qkv projection + rope (fp8 hi/lo DR) ========
        def emit_B_tile(ti):
            t0 = ti * 512
            xh = xt_pool.tile([P, CT, 512], f8, tag="xh", name="xh")
            xl = xt_pool.tile([P, CT, 512], f8, tag="xl", name="xl")
            for ch in range(4):
                nc.sync.dma_start(
                    xh[:, ch * 4 : (ch + 1) * 4, :],
                    xhi[ch * 512 : (ch + 1) * 512, t0 : t0 + 512].rearrange(
                        "(ct p) j -> p ct j", p=P
                    ),
                )
            if not first_tile_loaded[0]:
                first_tile_loaded[0] = True
                load_rest_weights()
            for ch in range(4):
                nc.sync.dma_start(
                    xl[:, ch * 4 : (ch + 1) * 4, :],
                    xlo[ch * 512 : (ch + 1) * 512, t0 : t0 + 512].rearrange(
                        "(ct p) j -> p ct j", p=P
                    ),
                )
            if ti == 0:
                nc.sync.dma_start(wvh_sb[:], wvhi.rearrange("(ct p) m -> p ct m", p=P))
                nc.sync.dma_start(wvl_sb[:], wvlo.rearrange("(ct p) m -> p ct m", p=P))
                nc.sync.dma_start(rotm_sb[:], rotmp[:])
                nc.vector.memset(v_sb[:, :, :, D : D + 1], float(ONESC))
            nc.sync.dma_start(cos_sb[:, t0 : t0 + 512], cos_t[:, t0 : t0 + 512])
            nc.sync.dma_start(sin_sb[:, t0 : t0 + 512], sin_t[:, t0 : t0 + 512])
            if ti == 1:
                nc.sync.dma_start(mask_sb[:], maskw[:])
                nc.sync.dma_start(id_sb[:], ident[:])
            # v chains are split into halves and woven into the rope rot
            # gaps: after rot_h0 the PE does half a v chain while the DVE/Pool
            # finish st_h1, so rot_h1 never stalls the PE.
            vstate = {"sub": 0, "vps": None, "piece": 0}

            def v_piece():
                sub = vstate["sub"]
                if sub >= 4:
                    return
                if vstate["piece"] == 0:
                    vstate["vps"] = pstile("ta", 2, name="vps")
                vps = vstate["vps"]
                base = vstate["piece"] * 12
                i = 0
                terms = [
                    (xsb, wsb, c)
                    for xsb, wsb in ((xh, wvh_sb), (xl, wvh_sb), (xh, wvl_sb))
                    for c in range(0, CT, 2)
                ]
                for i in range(base, base + 12):
                    xsb, wsb, c = terms[i]
                    nc.tensor.matmul(
                        vps[:, 0:256],
                        xsb[:, c : c + 2, sub * P : (sub + 1) * P],
                        wsb[:, c : c + 2, :],
                        start=(i == 0),
                        stop=(i == 23),
                        perf_mode=DR,
                    )
                if vstate["piece"] == 1:
                    tt = ti * 4 + sub
                    for h in range(HPC):
                        nc.vector.tensor_copy(
                            v_sb[:, tt, h, 0:D], vps[:, h * P : (h + 1) * P]
                        )
                    vstate["sub"] += 1
                vstate["piece"] ^= 1

            # q,k columns: out^T orientation -> [col128, tok256].
            for ci in range(4):
                hh = ci % HPC
                dstT = qT_sb if ci < HPC else kT_sb
                sts = []
                for half in range(2):
                    j0 = half * 256
                    ps = pstile("ta", 2, name="psqk")
                    i = 0
                    for wsb, xsb in ((wqkh_sb, xh), (wqkl_sb, xh), (wqkh_sb, xl)):
                        for c in range(0, CT, 2):
                            nc.tensor.matmul(
                                ps[:, 0:256],
                                wsb[:, c : c + 2, ci * P : (ci + 1) * P],
                                xsb[:, c : c + 2, j0 : j0 + 256],
                                start=(i == 0),
                                stop=(i == 23),
                                perf_mode=DR,
                            )
                            i += 1
                    st = rope_pool.tile([P, 256], f16, tag=f"st{half}", name="st")
                    if half == 0:
                        nc.vector.tensor_copy(st[:], ps[:, 0:256])
                    else:
                        nc.scalar.copy(st[:], ps[:, 0:256])
                    sts.append(st)
                for half in range(2):
                    j0 = half * 256
                    st = sts[half]
                    ps2 = pstile("tb", 2, name="psrot")
                    nc.tensor.matmul(
                        ps2[:, 0:256], rotm_sb[:], st[:], start=True, stop=True
                    )
                    t1 = rope_pool.tile([P, 256], f16, tag=f"t1{half}", name="t1")
                    t2 = rope_pool.tile([P, 256], f16, tag=f"t2{half}", name="t2")
                    nc.vector.tensor_mul(t1[:], st[:], cos_sb[:, t0 + j0 : t0 + j0 + 256])
                    nc.vector.tensor_mul(
                        t2[:], ps2[:, 0:256], sin_sb[:, t0 + j0 : t0 + j0 + 256]
                    )
                    nc.vector.tensor_add(
                        dstT[:, hh, t0 + j0 : t0 + j0 + 256], t1[:], t2[:]
                    )
                    if half == 0:
                        v_piece()
                v_piece()

        # ======== phase D chain supply (woven into C) ========
        dq = []  # pending (tt, cc, sub) projection chains
        ysb_map = {}

        d_alt = [0]

        def emit_d_chain(alt=False):
            if not dq:
                return False
            tt, cc, sub = dq.pop(0)
            if sub == 0:
                ysb_map[(tt, cc)] = ysb_pool.tile(
                    [P, 512], f16, tag="ysb", name="ysb"
                )
            ysb = ysb_map[(tt, cc)]
            c0 = cc * 512 + sub * 256
            if alt and d_alt[0] % 2 == 0:
                yps = pstile("ta", 2, name="yps2")
            else:
                yps = pstile("tb", 2, name="yps")
            d_alt[0] += 1
            i = 0
            for osb, wsb in ((oTh_sb, wph_sb), (oTl_sb, wph_sb), (oTh_sb, wpl_sb)):
                nc.tensor.matmul(
                    yps[:, 0:256],
                    osb[:, tt, :, :],
                    wsb[:, :, c0 : c0 + 256],
                    start=(i == 0),
                    stop=(i == 2),
                    perf_mode=DR,
                )
                i += 1
            act_share = 2 if alt else 8
            if (2 * cc + sub) % act_share == 0:
                nc.scalar.copy(ysb[:, sub * 256 : sub * 256 + 256], yps[:, 0:256])
            else:
                nc.vector.tensor_copy(ysb[:, sub * 256 : sub * 256 + 256], yps[:, 0:256])
            if sub == 1:
                nc.sync.dma_start(
                    y[tt * P : (tt + 1) * P, cc * 512 : (cc + 1) * 512], ysb[:]
                )
                del ysb_map[(tt, cc)]
            return True

        # ======== phase C: causal attention, one (b,h,qi) unit ========
        def emit_C(b, h, qi):
            toff = b * T
            q0 = toff + qi * 512
            ndiag0 = qi * 4
            nkt = ndiag0 + 4
            o_tiles = [
                pstile("co", 4, shape=(P, D + 1), name=f"o{_s}") for _s in range(4)
            ]

            def pv(pt_ap, kt, sub_lo):
                for s in range(sub_lo, 4):
                    nc.tensor.matmul(
                        o_tiles[s][:],
                        pt_ap(s),
                        v_sb[:, b * 16 + kt, h, :],
                        start=(kt == 0),
                        stop=(kt == ndiag0 + s),
                    )

            pend = []  # (ptd, kt, g) awaiting PV, emitted with lag 2

            def flush_pv():
                ptd_, kt_, g_ = pend.pop(0)
                pv(
                    lambda s, _p=ptd_, _g=g_: _p[:, s * P - _g : s * P - _g + P],
                    kt_,
                    max(kt_ - ndiag0, 0),
                )

            for kt in range(nkt):
                k0 = toff + kt * P
                gi = kt - ndiag0
                g = max(gi, 0) * P
                w = 512 - g
                sd = pstile("ta", 2, name="sd")
                nc.tensor.matmul(
                    sd[:, 0:w],
                    kT_sb[:, h, k0 : k0 + P],
                    qT_sb[:, h, q0 + g : q0 + 512],
                    start=True,
                    stop=True,
                )
                ptd = p_pool.tile([P, 512], f16, tag="pt", name="ptd")
                nc.scalar.activation(
                    ptd[:, 0:w], sd[:, 0:w], Exp,
                    scale=float(SCALE / (WSCALE * WSCALE)),
                )
                if gi >= 0:
                    if qi <= 1:
                        nc.vector.tensor_mul(
                            ptd[:, 0:w], ptd[:, 0:w], mask_sb[:, 384 : 384 + w]
                        )
                    else:
                        nc.gpsimd.tensor_mul(
                            ptd[:, 0:w], ptd[:, 0:w], mask_sb[:, 384 : 384 + w]
                        )
                emit_d_chain()
                if len(dq) > 56:
                    emit_d_chain()
                pend.append((ptd, kt, g))
                if len(pend) > 3:
                    flush_pv()
            while pend:
                emit_d_chain()
                flush_pv()
            # drain: normalize, transpose, fp8 hi/lo quantize; D chains fill
            # the DVE-latency bubbles between subs
            for sub in range(4):
                tt = b * 16 + qi * 4 + sub
                ot = o_tiles[sub]
                rtmp = osb_pool.tile([P, 1], f32, tag="rtmp", name="rtmp")
                nc.vector.reciprocal(rtmp[:], ot[:, D : D + 1])
                o_sb = osb_pool.tile([P, P], f16, tag="osb", name="osb")
                nc.vector.tensor_scalar_mul(o_sb[:], ot[:, 0:D], rtmp[:])
                emit_d_chain()
                if len(dq) > 24:
                    emit_d_chain()
                tp = pstile("tb", 2, shape=(P, P), dt=f16, name="tp")
                nc.tensor.transpose(tp[:], o_sb[:], id_sb[:])
                oT16 = osb_pool.tile([P, P], f16, tag="oT16", name="oT16")
                nc.vector.tensor_copy(oT16[:], tp[:])
                nc.gpsimd.tensor_copy(oTh_sb[:, tt, h, :], oT16[:])
                nc.gpsimd.tensor_sub(oTl_sb[:, tt, h, :], oT16[:], oTh_sb[:, tt, h, :])
                emit_d_chain()
                if len(dq) > 24:
                    emit_d_chain()

        # ======== schedule ========
        for ti in range(TOK512):
            emit_B_tile(ti)
        nc.sync.dma_start(wph_sb[:], wphi.rearrange("(h p) m -> p h m", p=P))
        nc.sync.dma_start(wpl_sb[:], wplo.rearrange("(h p) m -> p h m", p=P))
        for b in range(B):
            for qi in range(NQ):
                emit_C(b, 0, qi)
                emit_C(b, 1, qi)
                dq.extend(
                    (b * 16 + qi * 4 + s, cc, sub)
                    for s in range(4)
                    for cc in range(4)
                    for sub in range(2)
                )
        while emit_d_chain(alt=True):
            pass

    nc.compile()
    return nc


def _host_inputs(x, Wqkv, Wproj):
    """Build per-core device input maps (host-side sharding + fp8 hi/lo)."""
    import ml_dtypes

    NP8 = ml_dtypes.float8_e4m3

    def hilo(a):
        hi = a.astype(NP8)
        lo = (a - hi.astype(np.float32)).astype(NP8)
        return np.ascontiguousarray(hi), np.ascontiguousarray(lo)

    xTf = np.ascontiguousarray(x.reshape(NTOK, C).T).astype(np.float32)
    xhi, xlo = hilo(xTf)

    invf = 1.0 / (10000.0 ** (np.arange(0, D, 2, dtype=np.float32) / D))
    freqs = np.arange(T, dtype=np.float32)[:, None] * invf[None, :]  # [T, 64]
    cos = np.cos(freqs).astype(np.float32).T  # [64, T]
    sin = np.sin(freqs).astype(np.float32).T
    cos_t = np.tile(np.concatenate([cos, cos], axis=0), (1, B)).astype(np.float16)
    sin_t = np.tile(np.concatenate([-sin, sin], axis=0), (1, B)).astype(np.float16)

    ii = np.arange(P)[:, None]
    mm = np.arange(1024)[None, :]
    maskw = (mm >= ii + 384).astype(np.float16)
    ident = np.eye(P, dtype=np.float16)
    rotm = np.zeros((P, P), dtype=np.float16)
    rotm[(np.arange(P) + 64) % P, np.arange(P)] = 1.0

    in_maps = []
    for c in range(NCORES):
        h0 = c * HPC * D
        wqk_c = np.concatenate(
            [Wqkv[:, h0 : h0 + HPC * D], Wqkv[:, C + h0 : C + h0 + HPC * D]], axis=1
        ).astype(np.float32) * WSCALE
        wv_c = Wqkv[:, 2 * C + h0 : 2 * C + h0 + HPC * D].astype(np.float32) * WSCALE
        wp_c = Wproj[h0 : h0 + HPC * D, :].astype(np.float32) * WSCALE
        wqkhi, wqklo = hilo(wqk_c)
        wvhi, wvlo = hilo(wv_c)
        wphi, wplo = hilo(wp_c)
        in_maps.append(
            {
                "xhi": xhi,
                "xlo": xlo,
                "wqkhi": wqkhi,
                "wqklo": wqklo,
                "wvhi": wvhi,
                "wvlo": wvlo,
                "wphi": wphi,
                "wplo": wplo,
                "cos_t": cos_t,
                "sin_t": sin_t,
                "maskw": maskw,
                "ident": ident,
                "rotm": rotm,
            }
        )
    return in_maps


def kernel(x, Wqkv, Wproj, _trace=False):
    global _compiled
    x = np.asarray(x, dtype=np.float32)
    Wqkv = np.asarray(Wqkv, dtype=np.float32)
    Wproj = np.asarray(Wproj, dtype=np.float32)

    from concourse.bass_utils import run_bass_kernel_spmd

    if _compiled is None:
        _compiled = _build_bass()
    nc = _compiled

    in_maps = _host_inputs(x, Wqkv, Wproj)
    res = run_bass_kernel_spmd(nc, in_maps, list(range(NCORES)), trace=_trace)
    out = np.zeros((NTOK, C), dtype=np.float32)
    for r in res.results:
        out += r["y"].astype(np.float32)
    out /= YDIV
    kernel._last_result = res
    return out.reshape(B, T, C)


# revision 26
# speedup vs baseline: 1.1300x; 1.0067x over previous
"""Causal self-attention (B=2, T=2048, C=2048, H=16, rope) on 8 trn2 cores.

Sharding: tensor-parallel over heads (2 heads/core); host sums the 8
row-parallel partial output projections.

v2: fp8 (e4m3) hi/lo-split DoubleRow matmuls for the QKV and output
projections (3-term x_hi*w_hi + x_lo*w_hi + x_hi*w_lo, fp32 PSUM accum,
~1.4e-3 rel err), attention core in fp16. Weights pre-scaled x32 on host
so fp8 residuals stay in normal range; output rescaled on host.
Engine routing (gpsimd/Pool cannot touch PSUM): Act = exp + one st copy
per ci + 1/8 of y copies; DVE = rope muls, PSUM drains (st/recip/
normalize/oT/y copies); Pool = SBUF-only work (causal mask muls, O hi/lo
fp8 quantize). Schedule: B tiles run standalone (PE-dense, v chains
woven into rope-rot latency gaps); attention units pipeline S two k-tiles
ahead of PV (lag-3 flush) and weave output-projection chains into the
exp-latency and drain bubbles; trailing projections alternate PSUM rings.
PSUM banks: ta(2)=qk/v/S/… tb(2)=rot/yps/transposes, co(4)=O accum.
"""

import sys

for _p in ("/opt/trn_rl_repo",):
    if _p not in sys.path:
        sys.path.append(_p)

import numpy as np

# ---- problem constants (hardcoded per the task contract) ----
B, T, C, H = 2, 2048, 2048, 16
D = C // H  # 128
NCORES = 8
HPC = H // NCORES  # heads per core = 2
NTOK = B * T  # 4096
P = 128
CT = C // P  # 16 contraction tiles
TOK512 = NTOK // 512  # 8
NQ = T // 512  # q-tiles per unit = 4
TT = NTOK // P  # 32 token 128-tiles
SCALE = 1.0 / np.sqrt(D)
WSCALE = 32.0  # host pre-scale on all weights (fp8 residual range)
ONESC = 8.0  # ones-column value: o_sb = (32/ONESC) * O_normalized
YDIV = WSCALE * WSCALE / ONESC  # host divides y by this

_compiled = None

KNOBS = {"pt": 8, "ysb": 12, "rope": 4, "osb": 8, "xt": 2, "dfill": 4}


def _build_bass():
    import concourse.bacc as bacc
    import concourse.mybir as mybir
    import concourse.tile as tile
    from contextlib import ExitStack

    f16 = mybir.dt.float16
    f32 = mybir.dt.float32
    f8 = mybir.dt.float8e4
    DR = mybir.MatmulPerfMode.DoubleRow
    Exp = mybir.ActivationFunctionType.Exp

    nc = bacc.Bacc()

    xhi = nc.declare_dram_parameter("xhi", [C, NTOK], f8, isOutput=False)
    xlo = nc.declare_dram_parameter("xlo", [C, NTOK], f8, isOutput=False)
    wqkhi = nc.declare_dram_parameter("wqkhi", [C, 2 * HPC * D], f8, isOutput=False)
    wqklo = nc.declare_dram_parameter("wqklo", [C, 2 * HPC * D], f8, isOutput=False)
    wvhi = nc.declare_dram_parameter("wvhi", [C, HPC * D], f8, isOutput=False)
    wvlo = nc.declare_dram_parameter("wvlo", [C, HPC * D], f8, isOutput=False)
    wphi = nc.declare_dram_parameter("wphi", [HPC * D, C], f8, isOutput=False)
    wplo = nc.declare_dram_parameter("wplo", [HPC * D, C], f8, isOutput=False)
    cos_t = nc.declare_dram_parameter("cos_t", [P, NTOK], f16, isOutput=False)
    sin_t = nc.declare_dram_parameter("sin_t", [P, NTOK], f16, isOutput=False)
    maskw = nc.declare_dram_parameter("maskw", [P, 1024], f16, isOutput=False)
    ident = nc.declare_dram_parameter("ident", [P, P], f16, isOutput=False)
    rotmp = nc.declare_dram_parameter("rotm", [P, P], f16, isOutput=False)
    y = nc.declare_dram_parameter("y", [NTOK, C], f16, isOutput=True)

    with tile.TileContext(nc) as tc, ExitStack() as ctx:
        pers = ctx.enter_context(tc.tile_pool(name="pers", bufs=1))

        # ---- persistent SBUF tensors ----
        wqkh_sb = pers.tile([P, CT, 4 * P], f8)
        wqkl_sb = pers.tile([P, CT, 4 * P], f8)
        wvh_sb = pers.tile([P, CT, 2 * P], f8)
        wvl_sb = pers.tile([P, CT, 2 * P], f8)
        wph_sb = pers.tile([P, HPC, C], f8)
        wpl_sb = pers.tile([P, HPC, C], f8)
        cos_sb = pers.tile([P, NTOK], f16)
        sin_sb = pers.tile([P, NTOK], f16)
        mask_sb = pers.tile([P, 1024], f16)
        id_sb = pers.tile([P, P], f16)
        rotm_sb = pers.tile([P, P], f16)
        qT_sb = pers.tile([P, HPC, NTOK], f16)  # [d, h, tok] rope'd, 32x scale
        kT_sb = pers.tile([P, HPC, NTOK], f16)
        v_sb = pers.tile([P, TT, HPC, D + 1], f16)  # [tokmod, tt, h, D|ones*8]
        oTh_sb = pers.tile([P, TT, HPC, P], f8)  # [d, tt, h, tokmod] hi
        oTl_sb = pers.tile([P, TT, HPC, P], f8)  # lo residual

        xt_pool = ctx.enter_context(tc.tile_pool(name="xt", bufs=KNOBS["xt"]))
        rope_pool = ctx.enter_context(tc.tile_pool(name="rope", bufs=KNOBS["rope"]))
        p_pool = ctx.enter_context(tc.tile_pool(name="pt", bufs=KNOBS["pt"]))
        osb_pool = ctx.enter_context(tc.tile_pool(name="osb", bufs=KNOBS["osb"]))
        ysb_pool = ctx.enter_context(tc.tile_pool(name="ysb", bufs=KNOBS["ysb"]))
        # Single PSUM pool, 8 banks via per-tag bufs:
        #   ta(2): qk+v DR chains (B), S tiles + O transposes (C)
        #   tb(2): rope rot matmuls (B), yps projection chains (D)
        #   co(4): O accumulators (C)
        ps_pool = ctx.enter_context(tc.tile_pool(name="ps", bufs=1, space="PSUM"))

        def pstile(tag, bufs, shape=(P, 512), dt=f32, name="ps"):
            return ps_pool.tile(list(shape), dt, tag=tag, bufs=bufs, name=name)

        # startup DMAs ordered by first use
        for cth in range(2):
            nc.sync.dma_start(
                wqkh_sb[:, cth * 8 : (cth + 1) * 8, :],
                wqkhi[cth * 1024 : (cth + 1) * 1024, :].rearrange(
                    "(ct p) m -> p ct m", p=P
                ),
            )

        first_tile_loaded = [False]

        def load_rest_weights():
            for cth in range(2):
                nc.sync.dma_start(
                    wqkl_sb[:, cth * 8 : (cth + 1) * 8, :],
                    wqklo[cth * 1024 : (cth + 1) * 1024, :].rearrange(
                        "(ct p) m -> p ct m", p=P
                    ),
                )

        # ======== phase B: qkv projection + rope (fp8 hi/lo DR) ========
        def emit_B_tile(ti):
            t0 = ti * 512
            xh = xt_pool.tile([P, CT, 512], f8, tag="xh", name="xh")
            xl = xt_pool.tile([P, CT, 512], f8, tag="xl", name="xl")
            for ch in range(4):
                nc.sync.dma_start(
                    xh[:, ch * 4 : (ch + 1) * 4, :],
                    xhi[ch * 512 : (ch + 1) * 512, t0 : t0 + 512].rearrange(
                        "(ct p) j -> p ct j", p=P
                    ),
                )
            if not first_tile_loaded[0]:
                first_tile_loaded[0] = True
                load_rest_weights()
            for ch in range(4):
                nc.sync.dma_start(
                    xl[:, ch * 4 : (ch + 1) * 4, :],
                    xlo[ch * 512 : (ch + 1) * 512, t0 : t0 + 512].rearrange(
                        "(ct p) j -> p ct j", p=P
                    ),
                )
            if ti == 0:
                nc.sync.dma_start(wvh_sb[:], wvhi.rearrange("(ct p) m -> p ct m", p=P))
                nc.sync.dma_start(wvl_sb[:], wvlo.rearrange("(ct p) m -> p ct m", p=P))
                nc.sync.dma_start(rotm_sb[:], rotmp[:])
                nc.vector.memset(v_sb[:, :, :, D : D + 1], float(ONESC))
            nc.sync.dma_start(cos_sb[:, t0 : t0 + 512], cos_t[:, t0 : t0 + 512])
            nc.sync.dma_start(sin_sb[:, t0 : t0 + 512], sin_t[:, t0 : t0 + 512])
            if ti == 1:
                nc.sync.dma_start(mask_sb[:], maskw[:])
                nc.sync.dma_start(id_sb[:], ident[:])
            # v chains are split into halves and woven into the rope rot
            # gaps: after rot_h0 the PE does half a v chain while the DVE/Pool
            # finish st_h1, so rot_h1 never stalls the PE.
            vstate = {"sub": 0, "vps": None, "piece": 0}

            def v_piece():
                sub = vstate["sub"]
                if sub >= 4:
                    return
                if vstate["piece"] == 0:
                    vstate["vps"] = pstile("ta", 2, name="vps")
                vps = vstate["vps"]
                base = vstate["piece"] * 12
                i = 0
                terms = [
                    (xsb, wsb, c)
                    for xsb, wsb in ((xh, wvh_sb), (xl, wvh_sb), (xh, wvl_sb))
                    for c in range(0, CT, 2)
                ]
                for i in range(base, base + 12):
                    xsb, wsb, c = terms[i]
                    nc.tensor.matmul(
                        vps[:, 0:256],
                        xsb[:, c : c + 2, sub * P : (sub + 1) * P],
                        wsb[:, c : c + 2, :],
                        start=(i == 0),
                        stop=(i == 23),
                        perf_mode=DR,
                    )
                if vstate["piece"] == 1:
                    tt = ti * 4 + sub
                    for h in range(HPC):
                        nc.vector.tensor_copy(
                            v_sb[:, tt, h, 0:D], vps[:, h * P : (h + 1) * P]
                        )
                    vstate["sub"] += 1
                vstate["piece"] ^= 1

            # q,k columns: out^T orientation -> [col128, tok256].
            for ci in range(4):
                hh = ci % HPC
                dstT = qT_sb if ci < HPC else kT_sb
                sts = []
                for half in range(2):
                    j0 = half * 256
                    ps = pstile("ta", 2, name="psqk")
                    i = 0
                    for wsb, xsb in ((wqkh_sb, xh), (wqkl_sb, xh), (wqkh_sb, xl)):
                        for c in range(0, CT, 2):
                            nc.tensor.matmul(
                                ps[:, 0:256],
                                wsb[:, c : c + 2, ci * P : (ci + 1) * P],
                                xsb[:, c : c + 2, j0 : j0 + 256],
                                start=(i == 0),
                                stop=(i == 23),
                                perf_mode=DR,
                            )
                            i += 1
                    st = rope_pool.tile([P, 256], f16, tag=f"st{half}", name="st")
                    if half == 0:
                        nc.vector.tensor_copy(st[:], ps[:, 0:256])
                    else:
                        nc.scalar.copy(st[:], ps[:, 0:256])
                    sts.append(st)
                for half in range(2):
                    j0 = half * 256
                    st = sts[half]
                    ps2 = pstile("tb", 2, name="psrot")
                    nc.tensor.matmul(
                        ps2[:, 0:256], rotm_sb[:], st[:], start=True, stop=True
                    )
                    t1 = rope_pool.tile([P, 256], f16, tag=f"t1{half}", name="t1")
                    t2 = rope_pool.tile([P, 256], f16, tag=f"t2{half}", name="t2")
                    nc.vector.tensor_mul(t1[:], st[:], cos_sb[:, t0 + j0 : t0 + j0 + 256])
                    nc.vector.tensor_mul(
                        t2[:], ps2[:, 0:256], sin_sb[:, t0 + j0 : t0 + j0 + 256]
                    )
                    nc.vector.tensor_add(
                        dstT[:, hh, t0 + j0 : t0 + j0 + 256], t1[:], t2[:]
                    )
                    if half == 0:
                        v_piece()
                v_piece()

        # ======== phase D chain supply (woven into C) ========
        dq = []  # pending (tt, cc, sub) projection chains
        ysb_map = {}

        d_alt = [0]

        def emit_d_chain(alt=False):
            if not dq:
                return False
            tt, cc, sub = dq.pop(0)
            if sub == 0:
                ysb_map[(tt, cc)] = ysb_pool.tile(
                    [P, 512], f16, tag="ysb", name="ysb"
                )
            ysb = ysb_map[(tt, cc)]
            c0 = cc * 512 + sub * 256
            if alt and d_alt[0] % 2 == 0:
                yps = pstile("ta", 2, name="yps2")
            else:
                yps = pstile("tb", 2, name="yps")
            d_alt[0] += 1
            i = 0
            for osb, wsb in ((oTh_sb, wph_sb), (oTl_sb, wph_sb), (oTh_sb, wpl_sb)):
                nc.tensor.matmul(
                    yps[:, 0:256],
                    osb[:, tt, :, :],
                    wsb[:, :, c0 : c0 + 256],
                    start=(i == 0),
                    stop=(i == 2),
                    perf_mode=DR,
                )
                i += 1
            act_share = 2 if alt else 8
            if (2 * cc + sub) % act_share == 0:
                nc.scalar.copy(ysb[:, sub * 256 : sub * 256 + 256], yps[:, 0:256])
            else:
                nc.vector.tensor_copy(ysb[:, sub * 256 : sub * 256 + 256], yps[:, 0:256])
            if sub == 1:
                nc.sync.dma_start(
                    y[tt * P : (tt + 1) * P, cc * 512 : (cc + 1) * 512], ysb[:]
                )
                del ysb_map[(tt, cc)]
            return True

        # ======== phase C: causal attention, one (b,h,qi) unit ========
        def emit_C(b, h, qi):
            toff = b * T
            q0 = toff + qi * 512
            ndiag0 = qi * 4
            nkt = ndiag0 + 4
            o_tiles = [
                pstile("co", 4, shape=(P, D + 1), name=f"o{_s}") for _s in range(4)
            ]

            def pv(pt_ap, kt, sub_lo):
                for s in range(sub_lo, 4):
                    nc.tensor.matmul(
                        o_tiles[s][:],
                        pt_ap(s),
                        v_sb[:, b * 16 + kt, h, :],
                        start=(kt == 0),
                        stop=(kt == ndiag0 + s),
                    )

            pend = []  # (ptd, kt, g) awaiting PV, emitted with lag 2

            def flush_pv():
                ptd_, kt_, g_ = pend.pop(0)
                pv(
                    lambda s, _p=ptd_, _g=g_: _p[:, s * P - _g : s * P - _g + P],
                    kt_,
                    max(kt_ - ndiag0, 0),
                )

            for kt in range(nkt):
                k0 = toff + kt * P
                gi = kt - ndiag0
                g = max(gi, 0) * P
                w = 512 - g
                sd = pstile("ta", 2, name="sd")
                nc.tensor.matmul(
                    sd[:, 0:w],
                    kT_sb[:, h, k0 : k0 + P],
                    qT_sb[:, h, q0 + g : q0 + 512],
                    start=True,
                    stop=True,
                )
                ptd = p_pool.tile([P, 512], f16, tag="pt", name="ptd")
                nc.scalar.activation(
                    ptd[:, 0:w], sd[:, 0:w], Exp,
                    scale=float(SCALE / (WSCALE * WSCALE)),
                )
                if gi >= 0:
                    if qi <= 1:
                        nc.vector.tensor_mul(
                            ptd[:, 0:w], ptd[:, 0:w], mask_sb[:, 384 : 384 + w]
                        )
                    else:
                        nc.gpsimd.tensor_mul(
                            ptd[:, 0:w], ptd[:, 0:w], mask_sb[:, 384 : 384 + w]
                        )
                emit_d_chain()
                if len(dq) > 56:
                    emit_d_chain()
                pend.append((ptd, kt, g))
                if len(pend) > 3:
                    flush_pv()
            while pend:
                emit_d_chain()
                flush_pv()
            # drain: batched recips/norms stream on DVE, then per-sub
            # transpose + fp8 quantize with projection chains as PE filler
            rtmps = []
            for sub in range(4):
                rtmp = osb_pool.tile([P, 1], f32, tag=f"rtmp{sub}", name="rtmp")
                nc.vector.reciprocal(rtmp[:], o_tiles[sub][:, D : D + 1])
                rtmps.append(rtmp)
            o_sbs = []
            for sub in range(4):
                o_sb = osb_pool.tile([P, P], f16, tag=f"osb{sub}", name="osb")
                nc.vector.tensor_scalar_mul(o_sb[:], o_tiles[sub][:, 0:D], rtmps[sub][:])
                o_sbs.append(o_sb)
            for sub in range(4):
                tt = b * 16 + qi * 4 + sub
                emit_d_chain()
                if len(dq) > 24:
                    emit_d_chain()
                tp = pstile("tb", 2, shape=(P, P), dt=f16, name="tp")
                nc.tensor.transpose(tp[:], o_sbs[sub][:], id_sb[:])
                oT16 = osb_pool.tile([P, P], f16, tag=f"oT16{sub % 2}", name="oT16")
                nc.vector.tensor_copy(oT16[:], tp[:])
                nc.gpsimd.tensor_copy(oTh_sb[:, tt, h, :], oT16[:])
                nc.gpsimd.tensor_sub(oTl_sb[:, tt, h, :], oT16[:], oTh_sb[:, tt, h, :])
                emit_d_chain()
                if len(dq) > 24:
                    emit_d_chain()

        # ======== schedule ========
        for ti in range(TOK512):
            emit_B_tile(ti)
        nc.sync.dma_start(wph_sb[:], wphi.rearrange("(h p) m -> p h m", p=P))
        nc.sync.dma_start(wpl_sb[:], wplo.rearrange("(h p) m -> p h m", p=P))
        for b in range(B):
            for qi in range(NQ):
                emit_C(b, 0, qi)
                emit_C(b, 1, qi)
                dq.extend(
                    (b * 16 + qi * 4 + s, cc, sub)
                    for s in range(4)
                    for cc in range(4)
                    for sub in range(2)
                )
        while emit_d_chain(alt=True):
            pass

    nc.compile()
    return nc


def _host_inputs(x, Wqkv, Wproj):
    """Build per-core device input maps (host-side sharding + fp8 hi/lo)."""
    import ml_dtypes

    NP8 = ml_dtypes.float8_e4m3

    def hilo(a):
        hi = a.astype(NP8)
        lo = (a - hi.astype(np.float32)).astype(NP8)
        return np.ascontiguousarray(hi), np.ascontiguousarray(lo)

    xTf = np.ascontiguousarray(x.reshape(NTOK, C).T).astype(np.float32)
    xhi, xlo = hilo(xTf)

    invf = 1.0 / (10000.0 ** (np.arange(0, D, 2, dtype=np.float32) / D))
    freqs = np.arange(T, dtype=np.float32)[:, None] * invf[None, :]  # [T, 64]
    cos = np.cos(freqs).astype(np.float32).T  # [64, T]
    sin = np.sin(freqs).astype(np.float32).T
    cos_t = np.tile(np.concatenate([cos, cos], axis=0), (1, B)).astype(np.float16)
    sin_t = np.tile(np.concatenate([-sin, sin], axis=0), (1, B)).astype(np.float16)

    ii = np.arange(P)[:, None]
    mm = np.arange(1024)[None, :]
    maskw = (mm >= ii + 384).astype(np.float16)
    ident = np.eye(P, dtype=np.float16)
    rotm = np.zeros((P, P), dtype=np.float16)
    rotm[(np.arange(P) + 64) % P, np.arange(P)] = 1.0

    in_maps = []
    for c in range(NCORES):
        h0 = c * HPC * D
        wqk_c = np.concatenate(
            [Wqkv[:, h0 : h0 + HPC * D], Wqkv[:, C + h0 : C + h0 + HPC * D]], axis=1
        ).astype(np.float32) * WSCALE
        wv_c = Wqkv[:, 2 * C + h0 : 2 * C + h0 + HPC * D].astype(np.float32) * WSCALE
        wp_c = Wproj[h0 : h0 + HPC * D, :].astype(np.float32) * WSCALE
        wqkhi, wqklo = hilo(wqk_c)
        wvhi, wvlo = hilo(wv_c)
        wphi, wplo = hilo(wp_c)
        in_maps.append(
            {
                "xhi": xhi,
                "xlo": xlo,
                "wqkhi": wqkhi,
                "wqklo": wqklo,
                "wvhi": wvhi,
                "wvlo": wvlo,
                "wphi": wphi,
                "wplo": wplo,
                "cos_t": cos_t,
                "sin_t": sin_t,
                "maskw": maskw,
                "ident": ident,
                "rotm": rotm,
            }
        )
    return in_maps


def kernel(x, Wqkv, Wproj, _trace=False):
    global _compiled
    x = np.asarray(x, dtype=np.float32)
    Wqkv = np.asarray(Wqkv, dtype=np.float32)
    Wproj = np.asarray(Wproj, dtype=np.float32)

    from concourse.bass_utils import run_bass_kernel_spmd

    if _compiled is None:
        _compiled = _build_bass()
    nc = _compiled

    in_maps = _host_inputs(x, Wqkv, Wproj)
    res = run_bass_kernel_spmd(nc, in_maps, list(range(NCORES)), trace=_trace)
    out = np.zeros((NTOK, C), dtype=np.float32)
    for r in res.results:
        out += r["y"].astype(np.float32)
    out /= YDIV
    kernel._last_result = res
    return out.reshape(B, T, C)
